# revision 67
# baseline (speedup 1.0000x reference)
"""LocallyHierarchicalNet Trainium2 kernel.

Net: 10 locally-connected conv1d layers (kernel=stride=2, unshared weights
per position), B=128, C_in=3, H=256, D=1024, then mean + linear head.

Strategy (8 NeuronCores, SPMD):
  - Position-shard layers 0-6: core i owns output positions [i*64,(i+1)*64)
    of layer 0, which narrows to exactly 1 position at layer 6 with zero
    cross-core traffic (binary-tree locality).
  - AllGather the 8 layer-6 outputs (256x128 bf16 each) on-chip, then every
    core redundantly computes layers 7-9 + head (tiny).
  - All weights/activations stream and compute in bf16 (PSUM accumulates in
    fp32; final output fp32): halves HBM traffic vs fp32 and gets
    1 cyc/row matmul throughput at any free size.
  - Matmul layout: activations live as [C on partitions, B on free] per
    position.  Weights are the stationary operand (lhsT [K-chunk=128,
    O-half=128], host-pretransposed) and activations the moving operand
    (rhs [128, B]), so each position's output lands in PSUM already in
    [O, B] chain layout: one VectorE ReLU writes it back to SBUF with no
    PE transposes and no copy.
  - The gather path is laid out [cp, (b, ch)] so every DMA hop moves
    >=512B-contiguous runs (full DMA rate); layer 7 reads ch-strided views.
  - Discarded bridge matmuls keep the PE's busy-run contiguous across the
    gather window so layers 7-9 run at full clock (no pstate re-throttle).
  - Weights stream from HBM in multi-MB contiguous slabs, 4-deep
    buffered, alternating sync/gpsimd queues; x0+w0 ship as one fused
    input tensor and beta loads late, off the startup critical path.
"""

import sys

sys.path.insert(0, "/opt/trn_rl_repo")

import numpy as np

N_CORES = 8
B = 128
C_IN = 3
H = 256
OUT = 10

# per-core output positions per layer (layers 1..9 use 128-partition slabs)
NPOS = {1: 32, 2: 16, 3: 8, 4: 4, 5: 2, 6: 1, 7: 4, 8: 2, 9: 1}
# weight slab size (positions per DMA) per layer
SLAB = {1: 4, 2: 4, 3: 4, 4: 4, 5: 2, 6: 1, 7: 4, 8: 2, 9: 1}

_NC = None


def _build(sim_collective_stub=False):
    import concourse.bacc as bacc
    import concourse.mybir as mybir
    import concourse.tile as tile
    from concourse.masks import make_identity

    dt = mybir.dt
    f32 = dt.float32
    bf16 = dt.bfloat16
    Relu = mybir.ActivationFunctionType.Relu
    Copy = mybir.ActivationFunctionType.Copy

    nc = bacc.Bacc(
        "TRN2", target_bir_lowering=False, debug=False, num_devices=N_CORES
    )
    dma_engines = [nc.sync, nc.gpsimd]
    dma_rr = [0]

    def dma_q(out, in_):
        eng = dma_engines[dma_rr[0] % len(dma_engines)]
        dma_rr[0] += 1
        eng.dma_start(out, in_)

    xw0_d = nc.dram_tensor("xw0", [6, 64, B + H], bf16, kind="ExternalInput")
    w_d = {}
    for l in range(1, 10):
        w_d[l] = nc.dram_tensor(
            f"w{l}", [128, NPOS[l] * 1024], bf16, kind="ExternalInput"
        )
    beta_d = nc.dram_tensor("beta", [128, 2 * OUT], bf16, kind="ExternalInput")
    out_d = nc.dram_tensor("out", [B, OUT], f32, kind="ExternalOutput")

    with tile.TileContext(nc) as tc:
        with (
            tc.tile_pool(name="sb", bufs=1) as sb,
            tc.tile_pool(name="wp", bufs=4) as wp,
            tc.tile_pool(name="yp", bufs=4) as yp,
            tc.tile_pool(name="psp", bufs=3, space="PSUM") as psp,
            tc.tile_pool(name="ptp", bufs=2, space="PSUM") as ptp,
            tc.tile_pool(name="dram", bufs=1, space="DRAM") as dp,
        ):
            ident = sb.tile([128, 128], bf16, tag="ident", name="ident")
            make_identity(nc, ident)

            xw0 = sb.tile([6, 64, B + H], bf16, tag="x0", name="xw0_sb")
            nc.sync.dma_start(xw0[:], xw0_d[:])



            def relu_l0(pos, pt0, X1):
                if pos % 2 == 0:
                    nc.scalar.activation(X1[:, pos, :, :], pt0[:], Relu, scale=s3)
                else:
                    nc.vector.tensor_scalar(
                        X1[:, pos, :, :], pt0[:], s3, 0.0,
                        mybir.AluOpType.mult, mybir.AluOpType.max,
                    )

            # ---- layer 0: C_in=3, K=6; lhsT = w0 [6, O-half], rhs = x [6, B]
            # output written directly in chain layout [O, B] (no transpose).
            X1 = sb.tile([128, 64, 2, B], bf16, tag="xo", name="X1")
            s3 = 1.0 / (3.0**0.5)
            for pos in range(64):
                pt0 = ptp.tile([128, 2, B], f32, tag="pt0", name=f"p0_{pos}")
                for j in range(2):
                    nc.tensor.matmul(
                        pt0[:, j, :],
                        xw0[:, pos, B + j * 128 : B + (j + 1) * 128],
                        xw0[:, pos, 0:B],
                        start=True,
                        stop=True,
                    )
                relu_l0(pos, pt0, X1)


            def lc_layer(l, Xin, xtag, style="A", in_ileave=False, ws_pre=None):
                """One locally-connected layer l>=1 (C=256, K=512, O=256)."""
                n = NPOS[l]
                if xtag == "xown":
                    Xout = sb.tile([128, n, 2 * B], bf16, tag=xtag, name=f"X{l + 1}")
                else:
                    Xout = sb.tile([128, n, 2, B], bf16, tag=xtag, name=f"X{l + 1}")
                slab = SLAB[l]

                out_ileave = xtag == "xown"
                if style == "B":
                    # W stationary (lhsT [K-chunk, O-half]); PSUM lands in
                    # [O, B] chain layout; ReLU writes Xout directly.
                    for s in range(n // slab):
                        if ws_pre is None:
                            ws = wp.tile([128, slab * 1024], bf16, tag="ws", name=f"wsb{l}_{s}")
                            dma_q(ws[:], w_d[l][:, s * slab * 1024 : (s + 1) * slab * 1024])
                        else:
                            ws = ws_pre
                        for pp in range(slab):
                            p = s * slab + pp
                            ps = psp.tile([128, 2, B], f32, tag="ps", name=f"psb{l}_{p}")
                            for j in range(2):
                                for ci in range(4):
                                    kk, ch = ci >> 1, ci & 1
                                    off = ((pp * 2 + kk) * 2 + ch) * 2 + j
                                    rhs = (
                                        Xin.rearrange("c q (b ch) -> c q ch b", ch=2)[
                                            :, 2 * p + kk, ch, :
                                        ]
                                        if in_ileave
                                        else Xin[:, 2 * p + kk, ch, :]
                                    )
                                    nc.tensor.matmul(
                                        ps[:, j, :],
                                        ws[:, off * 128 : (off + 1) * 128],
                                        rhs,
                                        start=(ci == 0),
                                        stop=(ci == 3),
                                    )
                            xv = (
                                Xout.rearrange("c q (b ch) -> c q ch b", ch=2)[:, p, :, :]
                                if out_ileave
                                else Xout[:, p, :, :]
                            )
                            nc.vector.tensor_scalar(
                                xv, ps[:], 1.0 / 16.0, 0.0,
                                mybir.AluOpType.mult, mybir.AluOpType.max,
                            )
                    return Xout

                raise AssertionError("style A removed")

            X = X1
            for l, xtag in [(1, "xe"), (2, "xo2"), (3, "xe"), (4, "xo2"), (5, "xe")]:
                X = lc_layer(l, X, xtag)
            beta_sb = sb.tile([128, 2 * OUT], bf16, tag="beta", name="beta_sb")
            nc.sync.dma_start(beta_sb[:], beta_d[:])
            X = lc_layer(6, X, "xown", style="B")

            # ---- AllGather the single layer-6 output position across cores.
            # Layout [cp, (b, ch)]: every DMA hop moves 512B-contiguous runs
            # (the [ch,cp,b] layout would move 256B runs at half DMA rate).
            ag_in = dp.tile([128, 2 * B], bf16, name="ag_in")
            ag_out = dp.tile(
                [N_CORES * 128, 2 * B],
                bf16,
                addr_space="Local" if sim_collective_stub else "Shared",
                name="ag_out",
            )
            w789 = {
                l: sb.tile([128, NPOS[l] * 1024], bf16, tag=f"w{l}pre", name=f"w{l}pre")
                for l in (7, 8, 9)
            }
            nc.sync.dma_start(ag_in[:], X[:, 0, :])
            nc.sync.dma_start(w789[7][:, 0:2048], w_d[7][:, 0:2048])
            if sim_collective_stub:
                # timeline-sim only: stub the gather as a DMA (same
                # convention as the 156961ns baseline measurement).
                nc.sync.dma_start(ag_out[0:128, :], ag_in[:])
                nc.sync.dma_start(w789[7][:, 2048:4096], w_d[7][:, 2048:4096])
            else:
                nc.gpsimd.collective_compute(
                    "AllGather",
                    mybir.AluOpType.bypass,
                    replica_groups=[list(range(N_CORES))],
                    ins=[ag_in.opt()],
                    outs=[ag_out.opt()],
                )
                nc.sync.dma_start(w789[7][:, 2048:4096], w_d[7][:, 2048:4096])
            X7 = sb.tile([128, 8, 2 * B], bf16, tag="x7", name="X7")
            ag_view = ag_out.rearrange("(pos p) w -> p pos w", pos=8)
            nc.sync.dma_start(X7[:, 0:2, :], ag_view[:, 0:2, :])
            nc.sync.dma_start(w789[8][:], w_d[8][:])
            nc.sync.dma_start(X7[:, 2:8, :], ag_view[:, 2:8, :])
            nc.sync.dma_start(w789[9][:], w_d[9][:])

            X = X7
            X = lc_layer(7, X, "xo2", style="B", in_ileave=True, ws_pre=w789[7])
            X = lc_layer(8, X, "xe", style="B", ws_pre=w789[8])
            X = lc_layer(9, X, "xo2", style="B", ws_pre=w789[9])

            # ---- head: out[b, j] = sum_c X10[c, b] * beta[c, j] / 256
            ph = ptp.tile([128, OUT], f32, tag="pt", name="ph")
            for ch in range(2):
                nc.tensor.matmul(
                    ph[:],
                    X[:, 0, ch, :],
                    beta_sb[:, ch * OUT : (ch + 1) * OUT],
                    start=(ch == 0),
                    stop=(ch == 1),
                )
            ob = yp.tile([128, OUT], f32, tag="ob", name="ob")
            nc.scalar.copy(ob[:], ph[:])
            nc.sync.dma_start(out_d[:], ob[:])

    nc.compile()
    return nc


def _get_nc():
    global _NC
    if _NC is None:
        _NC = _build()
    return _NC


def _prep(inputs):
    import ml_dtypes

    bf16 = ml_dtypes.bfloat16

    x = np.asarray(inputs["x"], dtype=np.float32).astype(bf16)
    beta = np.asarray(inputs["beta"], dtype=np.float32).astype(bf16)
    ws = [np.asarray(inputs[f"w{l}"], dtype=np.float32).astype(bf16) for l in range(10)]

    # x (B,3,1024) -> (kk=2, c=3, d=512, b)
    xk = np.ascontiguousarray(x.reshape(B, 3, 512, 2).transpose(3, 1, 2, 0))
    # w0 (256,3,512,2) -> (kk, c, d, o)
    w0t = np.ascontiguousarray(ws[0].transpose(3, 1, 2, 0))
    # beta (256, 10) -> [128, (ch, 10)]
    betat = np.ascontiguousarray(
        (beta.astype(np.float32) / 256.0).astype(bf16).reshape(2, 128, OUT).transpose(1, 0, 2)
    ).reshape(128, 2 * OUT)

    # wl (256,256,dl,2) -> slab (cp=128, (pos, kk, ch, o=256))
    slabs = {}
    for l in range(1, 10):
        w = ws[l]
        dl = w.shape[2]
        wt = w.transpose(1, 2, 3, 0)  # (c, dl, kk, o)
        if l < 1:
            # plan A (W moving): (cp, (pos, kk, ch, o=256))
            wt = wt.reshape(2, 128, dl, 2, 256).transpose(1, 2, 3, 0, 4)
        else:
            # plan B (W stationary lhsT): (cp, (pos, kk, ch, j, oi=128))
            wt = wt.reshape(2, 128, dl, 2, 2, 128).transpose(1, 2, 3, 0, 4, 5)
        slabs[l] = np.ascontiguousarray(wt).reshape(128, dl * 1024)

    in_maps = []
    for i in range(N_CORES):
        xc = xk[:, :, i * 64 : (i + 1) * 64, :].reshape(6, 64, B)
        wc = w0t[:, :, i * 64 : (i + 1) * 64, :].reshape(6, 64, H)
        m = {
            "xw0": np.ascontiguousarray(np.concatenate([xc, wc], axis=2)),
            "beta": betat,
        }
        for l in range(1, 7):
            n = NPOS[l]
            m[f"w{l}"] = np.ascontiguousarray(
                slabs[l][:, i * n * 1024 : (i + 1) * n * 1024]
            )
        for l in range(7, 10):
            m[f"w{l}"] = slabs[l]
        in_maps.append(m)
    return in_maps


def _run(in_maps, trace=False):
    from concourse import bass_utils

    return bass_utils.run_bass_kernel_spmd(
        _get_nc(), in_maps, core_ids=list(range(N_CORES)), trace=trace
    )


def kernel(**inputs):
    res = _run(_prep(inputs))
    return np.asarray(res.results[0]["out"], dtype=np.float32)
